# revision 1
# baseline (speedup 1.0000x reference)
"""Bidirectional LSTM encoder (T=512, B=64, E=512, H=1024) on 8 TRN2 cores.

Strategy (data-parallel, direction-split): cores 0-3 run the forward
LSTM on batch quarters (B=16 each); cores 4-7 run the backward LSTM as
a forward scan over the time-reversed sequence with the backward
weights — one SPMD program, different per-core inputs. Matmul operands
are bf16 (PE 2x over f32), accumulation/state/outputs f32.

Per core: Phase A precomputes x-projections for all timesteps as one
GEMM (stationary Wih^T tiles, moving embedded tokens, fused bias at
PSUM eviction) into DRAM. Phase B runs the 512-step recurrence with
Whh^T resident in SBUF: per step, 32 gate-tiles x 8 K-chunks of
[128,128] stationary x [128,16] moving matmuls into one PSUM bank
(gates transposed layout [gate-rows, batch], so every non-PE engine
works across full 128 partitions), then DVE/ACT gate nonlinearities
and state update. Host does the embedding gather, layout prep, and
output assembly (hs = hs_f + reversed hs_b).
"""
import sys

sys.path.insert(0, "/opt/trn_rl_repo")

import numpy as np
import ml_dtypes

import concourse.bass as bass
import concourse.bacc as bacc
from concourse import mybir
from concourse.tile import TileContext

NPBF16 = ml_dtypes.bfloat16
F32 = mybir.dt.float32
BF16 = mybir.dt.bfloat16
AF = mybir.ActivationFunctionType

T, BATCH, E, H = 512, 64, 512, 1024
G = 4 * H
BL = 16  # batch per core (64 / 4 shards per direction group)


def _build_lstm(tc, outs, ins, T, B, E, H, unroll):
    nc = tc.nc
    G = 4 * H
    KE = E // 128
    KH = H // 128
    NT = G // 128
    NTOK = T * B
    NCHUNK = NTOK // 512

    embT, wihT, whhT, biasT = (ins["embT"], ins["wihT"], ins["whhT"],
                               ins["biasT"])
    hs = outs["hs"]
    xprojT = nc.dram_tensor("xprojT", [T, G, B], BF16, kind="Internal")

    with tc.tile_pool(name="wih", bufs=1) as wih_pool, \
         tc.tile_pool(name="bias", bufs=1) as bias_pool, \
         tc.tile_pool(name="emb", bufs=3) as emb_pool, \
         tc.tile_pool(name="xp_out", bufs=4) as xp_pool, \
         tc.tile_pool(name="psumA", bufs=4, space="PSUM") as psA, \
         tc.tile_pool(name="whh", bufs=1) as whh_pool, \
         tc.tile_pool(name="state", bufs=1) as st_pool, \
         tc.tile_pool(name="xp_in", bufs=4) as xpi_pool, \
         tc.tile_pool(name="gates", bufs=2) as g_pool, \
         tc.tile_pool(name="hout", bufs=3) as ho_pool, \
         tc.tile_pool(name="psumB", bufs=2, space="PSUM") as psB:

        # ---- Phase A: x-projection GEMM, bias fused at eviction ----
        wih_sb = wih_pool.tile([128, KE * G], BF16)
        for kc in range(KE):
            nc.sync.dma_start(
                wih_sb[:, kc * G:(kc + 1) * G],
                wihT[kc * 128:(kc + 1) * 128, :])
        bias_sb = bias_pool.tile([128, NT], F32)
        nc.sync.dma_start(bias_sb[:], biasT[:])

        spc = 512 // B
        for j in range(NCHUNK):
            emb_sb = emb_pool.tile([128, KE * 512], BF16)
            for kc in range(KE):
                nc.sync.dma_start(
                    emb_sb[:, kc * 512:(kc + 1) * 512],
                    embT[kc * 128:(kc + 1) * 128, j * 512:(j + 1) * 512])
            for ti in range(NT):
                psum = psA.tile([128, 512], F32)
                for kc in range(KE):
                    nc.tensor.matmul(
                        psum[:],
                        wih_sb[:, kc * G + ti * 128: kc * G + (ti + 1) * 128],
                        emb_sb[:, kc * 512:(kc + 1) * 512],
                        start=(kc == 0), stop=(kc == KE - 1))
                xp_sb = xp_pool.tile([128, 512], BF16)
                nc.scalar.activation(xp_sb[:], psum[:], AF.Identity,
                                     bias=bias_sb[:, ti:ti + 1])
                nc.sync.dma_start(
                    xprojT[j * spc:(j + 1) * spc,
                           ti * 128:(ti + 1) * 128, :]
                    .rearrange("s p b -> p s b"),
                    xp_sb[:].rearrange("p (s b) -> p s b", b=B))

        # ---- Phase B: recurrence, Whh^T resident in SBUF ----
        whh_sb = whh_pool.tile([128, KH * G], BF16)
        for kc in range(KH):
            nc.sync.dma_start(
                whh_sb[:, kc * G:(kc + 1) * G],
                whhT[kc * 128:(kc + 1) * 128, :])

        HB = KH * B
        c_sb = st_pool.tile([128, HB], F32)
        h_sb = st_pool.tile([128, HB], BF16)
        nc.vector.memset(c_sb[:], 0.0)
        nc.vector.memset(h_sb[:], 0.0)

        def step(idx):
            xp_sb = xpi_pool.tile([128, NT * B], BF16)
            nc.sync.dma_start(
                xp_sb[:].rearrange("p (ti b) -> p ti b", b=B),
                xprojT[idx].rearrange("(ti p) b -> p ti b", p=128))
            psum_g = psB.tile([128, NT * B], F32)
            for ti in range(NT):
                for kc in range(KH):
                    nc.tensor.matmul(
                        psum_g[:, ti * B:(ti + 1) * B],
                        whh_sb[:, kc * G + ti * 128: kc * G + (ti + 1) * 128],
                        h_sb[:, kc * B:(kc + 1) * B],
                        start=(kc == 0), stop=(kc == KH - 1))
            gact = g_pool.tile([128, NT * B], F32)
            gates = g_pool.tile([128, NT * B], F32)
            nc.vector.tensor_add(gates[:], psum_g[:], xp_sb[:])
            io_, fo_, go_, oo_ = 0, KH * B, 2 * KH * B, 3 * KH * B
            nc.scalar.activation(gact[:, io_:go_], gates[:, io_:go_],
                                 AF.Sigmoid)
            nc.scalar.activation(gact[:, go_:oo_], gates[:, go_:oo_], AF.Tanh)
            nc.scalar.activation(gact[:, oo_:], gates[:, oo_:], AF.Sigmoid)
            ig = ho_pool.tile([128, HB], F32)
            fc = ho_pool.tile([128, HB], F32)
            nc.vector.tensor_mul(ig[:], gact[:, io_:fo_], gact[:, go_:oo_])
            nc.vector.tensor_mul(fc[:], gact[:, fo_:go_], c_sb[:])
            nc.vector.tensor_add(c_sb[:], ig[:], fc[:])
            tanhc = ho_pool.tile([128, HB], F32)
            nc.scalar.activation(tanhc[:], c_sb[:], AF.Tanh)
            hf = ho_pool.tile([128, HB], F32)
            nc.vector.tensor_mul(hf[:], gact[:, oo_:], tanhc[:])
            nc.vector.tensor_copy(h_sb[:], hf[:])
            nc.sync.dma_start(
                hs[idx].rearrange("k p b -> p k b"),
                hf[:].rearrange("p (k b) -> p k b", b=B))

        if unroll >= T:
            for t in range(T):
                step(t)
        else:
            with tc.For_i(0, T, unroll) as t0:
                for u in range(unroll):
                    step(t0 + u)


_NC_CACHE = {}


def _get_nc(unroll=8):
    if unroll not in _NC_CACHE:
        nc = bacc.Bacc("TRN2", num_devices=8)
        embT = nc.dram_tensor("embT", [E, T * BL], BF16,
                              kind="ExternalInput")
        wihT = nc.dram_tensor("wihT", [E, G], BF16, kind="ExternalInput")
        whhT = nc.dram_tensor("whhT", [H, G], BF16, kind="ExternalInput")
        biasT = nc.dram_tensor("biasT", [128, G // 128], F32,
                               kind="ExternalInput")
        hs = nc.dram_tensor("hs", [T, H // 128, 128, BL], F32,
                            kind="ExternalOutput")
        ins = {"embT": embT[:], "wihT": wihT[:], "whhT": whhT[:],
               "biasT": biasT[:]}
        outs = {"hs": hs[:]}
        with TileContext(nc) as tc:
            _build_lstm(tc, outs, ins, T=T, B=BL, E=E, H=H, unroll=unroll)
        nc.compile()
        _NC_CACHE[unroll] = nc
    return _NC_CACHE[unroll]


def _prep_core_inputs(x_shard, Wih, Whh, bih, bhh):
    embT = np.ascontiguousarray(x_shard.reshape(T * BL, E).T).astype(NPBF16)
    wihT = np.ascontiguousarray(np.asarray(Wih).T).astype(NPBF16)
    whhT = np.ascontiguousarray(np.asarray(Whh).T).astype(NPBF16)
    bias = (np.asarray(bih) + np.asarray(bhh)).astype(np.float32)
    biasT = np.ascontiguousarray(bias.reshape(G // 128, 128).T)
    return {"embT": embT, "wihT": wihT, "whhT": whhT, "biasT": biasT}


def kernel(source, emb_w, Wih_f, Whh_f, bih_f, bhh_f,
           Wih_b, Whh_b, bih_b, bhh_b, trace=False, unroll=8):
    source = np.asarray(source)
    emb = np.asarray(emb_w, dtype=np.float32).copy()
    emb[0] = 0.0  # padding_idx=0
    x = emb[source]            # [T, BATCH, E]
    xr = x[::-1]

    in_maps = []
    for c in range(4):
        in_maps.append(_prep_core_inputs(
            x[:, c * BL:(c + 1) * BL], Wih_f, Whh_f, bih_f, bhh_f))
    for c in range(4):
        in_maps.append(_prep_core_inputs(
            xr[:, c * BL:(c + 1) * BL], Wih_b, Whh_b, bih_b, bhh_b))

    nc = _get_nc(unroll)
    from concourse import bass_utils
    res = bass_utils.run_bass_kernel_spmd(
        nc, in_maps, core_ids=list(range(8)), trace=trace)

    out = np.empty((BATCH, T, H), np.float32)
    for c in range(4):
        f = res.results[c]["hs"].reshape(T, H, BL)
        b = res.results[4 + c]["hs"].reshape(T, H, BL)[::-1]
        out[c * BL:(c + 1) * BL] = (f + b).transpose(2, 0, 1)
    kernel.last_results = res
    return out
